# revision 1
# baseline (speedup 1.0000x reference)
"""Trainium2 Bass kernel for nn_Attention2 (dense transformer block with
softmax over the heads axis).

Computation per (n, t) batch b (B = n*t = 4096 total, X_b = x[n,:,t,:].T is
[vv=25, c=512]):
    qkv = X_b @ w_qkv.T, split into q,k,v heads [h=8, 25, hd=64]
    s[h,i,j] = (q[h,i,:] . k[h,j,:]) / 8      (scale folded into w_q on host)
    p = softmax over h (axis 0)
    o[h,i,:] = sum_j p[h,i,j] v[h,j,:]  -> [25, 512] -> @ w_proj.T
    out[n,:,t,:] = result.T

Sharding: data-parallel over n, 2 n-values (512 batches) per core, 8 cores.

Layout trick: x[n, :, t, :] is naturally X_b^T ([c, vv], c on partitions), so
the qkv and proj matmuls run as weight-stationary batched GEMMs with the
batch dim fused into the moving free dim (16 batches -> N=400).  v is
computed in V[j, c'] layout (j on partitions) via x-slab-stationary matmuls
so the attention-core matmuls need no transposes.  fp32r (1 cycle/row vs 4
for fp32, ~1.6e-4 rel err) is used for the three big GEMMs; the tiny
attention matmuls run fp32 packed onto the PE array with tile_position.
"""
import numpy as np
import concourse.bass as bass
import concourse.mybir as mybir
import concourse.tile as tile
from concourse.bass_utils import run_bass_kernel_spmd
from concourse.vector_clock import ScopedClock, VectorClock

F32 = mybir.dt.float32
F32R = mybir.dt.float32r
F16 = mybir.dt.float16

N_CORES = 8
NN_PER_CORE = 2        # n values per core
T = 256
VV = 25
C = 512
H = 8
HD = 64
TG = 16                # t values (batches) per group
NGROUPS = NN_PER_CORE * (T // TG)   # 32 groups per core
NB = TG * VV           # 400 moving columns per group


def _split_drain_and_barrier(self, tick_clock, wait_clock):
    # walrus caps sync-wait commands at 1 for CTRL_NO; split the kernel-tail
    # drain into one drain per pending proc.
    vc = tick_clock.global_clock
    n = len(vc)
    for i in range(n):
        if vc[i] == 0:
            continue
        sub = VectorClock([vc[j] if j == i else 0 for j in range(n)])
        d = self.nc.sync.drain()
        wait_clock.add_sem_waits(d.ins, ScopedClock({None: sub}))
    self.nc.all_engine_barrier()
    assert self.sems is not None
    popped = self.nc._tile_sem_poison_stack.pop()
    assert popped is self._sem_poison
    self.nc.clear_and_free_semaphores(list(self.sems.allocated().values()))
    self.nc.all_engine_barrier()


tile.TileContext._drain_and_barrier = _split_drain_and_barrier


def split_excess_waits(nc, limit=1):
    """walrus codegen allows very few sync-wait commands per instruction
    (1 for matmul/drain/DMA structs).  Move excess waits onto same-engine
    NoOp carriers inserted just before the instruction — same semantics,
    since each engine executes its queue in order."""
    k = 0
    for fn in nc.m.functions:
        for bb in fn.blocks:
            out = []
            for ins in bb.instructions:
                si = ins.sync_info
                waits = list(si.on_wait) if si is not None and si.on_wait else []
                if len(waits) > limit:
                    keep = waits[-limit:]
                    for w in waits[:-limit]:
                        nop = mybir.InstNoOp(
                            name=f"WC-{k}", ins=[], outs=[], engine=ins.engine
                        )
                        k += 1
                        nop.sync_info = mybir.SyncInfo(on_wait=[w], on_update=[])
                        out.append(nop)
                    si.on_wait = keep
                out.append(ins)
            bb.instructions[:] = out
    return k


def build_nc():
    nc = bass.Bass()
    X = nc.declare_dram_parameter("x", [NN_PER_CORE, C, T, VV], F16, isOutput=False)
    WQK = nc.declare_dram_parameter("wqkT", [C, 2 * C], F16, isOutput=False)
    WV = nc.declare_dram_parameter("wvT", [C, C], F16, isOutput=False)
    WP = nc.declare_dram_parameter("wprojT", [C, C], F16, isOutput=False)
    Y = nc.declare_dram_parameter("y", [NN_PER_CORE, C, T, VV], F32, isOutput=True)

    with tile.TileContext(nc) as tc:
        with (
            tc.tile_pool(name="consts", bufs=1) as consts,
            tc.tile_pool(name="perg", bufs=2) as perg,
            tc.tile_pool(name="pers", bufs=6) as pers,
            tc.tile_pool(name="pbig", bufs=2, space="PSUM") as pbig,
            tc.tile_pool(name="psmall", bufs=1, space="PSUM") as psmall,
        ):
            # ---- load + fp32r-convert the weights (DVE produces every
            # matmul operand so each matmul carries a single wait) ----
            wqk_r, wv_r, wp_r = [], [], []
            for kc in range(4):
                r0 = consts.tile([128, 2 * C], F16, tag=f"wqkr{kc}")
                nc.sync.dma_start(out=r0, in_=WQK[kc * 128:(kc + 1) * 128, :])
                wqk_r.append(r0)
                r1 = consts.tile([128, C], F16, tag=f"wvr{kc}")
                nc.sync.dma_start(out=r1, in_=WV[kc * 128:(kc + 1) * 128, :])
                wv_r.append(r1)
                r2 = consts.tile([128, C], F16, tag=f"wpr{kc}")
                nc.sync.dma_start(out=r2, in_=WP[kc * 128:(kc + 1) * 128, :])
                wp_r.append(r2)

            for g in range(NGROUPS):
                nn = g // (T // TG)
                t0 = (g % (T // TG)) * TG

                # ---- load x slab: 4 c-chunks of [128, 16, 25] ----
                xp = []
                for kc in range(4):
                    xq = perg.tile([128, TG, 32], F16, tag=f"xp{kc}")
                    nc.sync.dma_start(
                        out=xq[:, :, 0:VV],
                        in_=X[nn, kc * 128:(kc + 1) * 128, t0:t0 + TG, :],
                    )
                    xp.append(xq)

                # ---- q^T / k^T: out chunk m rows = c' = h*64+d (heads
                # 2m, 2m+1), cols = (b, i);  m 0-3 = q^T, 4-7 = k^T ----
                qkT = []
                for m in range(8):
                    pq = pbig.tile([128, NB], F32, tag="big")
                    for kc in range(4):
                        nc.tensor.matmul(
                            pq[:],
                            wqk_r[kc][:, m * 128:(m + 1) * 128],
                            xp[kc][:, :, 0:VV],
                            start=(kc == 0), stop=(kc == 3),
                        )
                    qc = perg.tile([128, NB], F16, tag=f"qkT{m}")
                    nc.vector.tensor_copy(qc[:, 0:NB // 2], pq[:, 0:NB // 2])
                    nc.vector.tensor_copy(qc[:, NB // 2:], pq[:, NB // 2:])
                    qkT.append(qc)

                oT = perg.tile([128, 4, NB], F16, tag="oT", name="oT")

                for sub in range(4):
                    bcol0 = sub * 4 * VV

                    pv = pbig.tile([128, C], F32, tag="big", name="pv")
                    for kc in range(4):
                        nc.tensor.matmul(
                            pv[:],
                            xp[kc][:, sub * 4:sub * 4 + 4, :],
                            wv_r[kc][:],
                            start=(kc == 0), stop=(kc == 3),
                        )
                    v2 = [pers.tile([64, C], F16, tag=f"v2{q}", name=f"v2{q}") for q in range(2)]
                    for q in range(2):
                        nc.scalar.activation(
                            v2[q][:, :], pv[q * 64:(q + 1) * 64, :],
                            mybir.ActivationFunctionType.Copy,
                        )

                    psm = [
                        psmall.tile([128, 4, VV], F32, tag=f"psm{par}", name=f"psm{par}", bufs=2)
                        for par in range(2)
                    ]
                    for h in range(H):
                        m, par, r0 = h // 2, h % 2, (h % 2) * 64
                        for b4 in range(4):
                            bcol = bcol0 + b4 * VV
                            nc.tensor.matmul(
                                psm[par][b4 * 32:b4 * 32 + 25, m, :],
                                qkT[4 + m][r0:r0 + 64, bcol:bcol + VV],
                                qkT[m][r0:r0 + 64, bcol:bcol + VV],
                                start=True, stop=True,
                                tile_position=(r0, b4 * 32),
                            )

                    e_t = perg.tile([128, VV, H], F32, tag="e_t", bufs=3)
                    for par in range(2):
                        nc.scalar.activation(
                            e_t[:, :, par::2],
                            psm[par][:].rearrange("p m i -> p i m"),
                            mybir.ActivationFunctionType.Exp,
                        )
                    D = perg.tile([128, VV], F32, tag="D", bufs=3)
                    nc.vector.reduce_sum(out=D[:], in_=e_t[:], axis=mybir.AxisListType.X)
                    rD = perg.tile([128, VV], F32, tag="rD", bufs=3)
                    nc.vector.reciprocal(rD[:], D[:])
                    p2 = [pers.tile([64, VV, H], F16, tag=f"p2{q}", name=f"p2{q}") for q in range(2)]
                    for q in range(2):
                        nc.vector.tensor_mul(
                            p2[q][:],
                            e_t[q * 64:(q + 1) * 64, :, :],
                            rD[q * 64:(q + 1) * 64, :]
                            .unsqueeze(2).broadcast_to([64, VV, H]),
                        )

                    po = [
                        psmall.tile([128, 4, 2 * VV], F32, tag=f"po{e}", name=f"po{e}")
                        for e in range(2)
                    ]
                    for b4 in range(4):
                        q, e = b4 // 2, b4 % 2
                        for h in range(H):
                            m, c0 = h // 2, (h % 2) * 64
                            nc.tensor.matmul(
                                po[e][c0:c0 + 64, m, q * VV:(q + 1) * VV],
                                v2[q][e * 32:e * 32 + 25, h * HD:(h + 1) * HD],
                                p2[q][e * 32:e * 32 + 25, :, h],
                                start=True, stop=True,
                                tile_position=(e * 32, c0),
                            )
                    for e in range(2):
                        dst = oT[:].rearrange(
                            "p m (b i) -> p m b i", i=VV
                        )[:, :, sub * 4 + e:sub * 4 + e + 3:2, :]
                        nc.vector.tensor_copy(
                            dst, po[e][:].rearrange(
                                "p m (b i) -> p m b i", i=VV
                            )
                        )

                # ---- proj: final^T[co, (b,i)] ----
                for co in range(4):
                    pf = pbig.tile([128, NB], F32, tag="big")
                    for kc in range(4):
                        nc.tensor.matmul(
                            pf[:],
                            wp_r[kc][:, co * 128:(co + 1) * 128],
                            oT[:, kc, :],
                            start=(kc == 0), stop=(kc == 3),
                        )
                    fin = perg.tile([128, NB], F32, tag=f"fin{co}")
                    nc.scalar.activation(
                        fin[:], pf[:], mybir.ActivationFunctionType.Copy,
                    )
                    nc.sync.dma_start(
                        out=Y[nn, co * 128:(co + 1) * 128, t0:t0 + TG, :],
                        in_=fin[:].rearrange("p (t v) -> p t v", t=TG),
                    )
    return nc


LAST_RESULT = {}


def kernel(x: np.ndarray, w_qkv: np.ndarray, w_proj: np.ndarray,
           _trace: bool = False) -> np.ndarray:
    n, c, t, vv = x.shape
    assert (n, c, t, vv) == (16, 512, 256, 25)
    scale = np.float32((c // H) ** -0.5)

    wq = w_qkv[:c] * scale
    wk = w_qkv[c:2 * c]
    wv = w_qkv[2 * c:]
    wqkT = np.ascontiguousarray(np.concatenate([wq, wk], axis=0).T.astype(np.float16))
    wvT = np.ascontiguousarray(wv.T.astype(np.float16))
    wprojT = np.ascontiguousarray(w_proj.T.astype(np.float16))

    nc = build_nc()
    split_excess_waits(nc)
    in_maps = []
    for core in range(N_CORES):
        shard = np.ascontiguousarray(
            x[core * NN_PER_CORE:(core + 1) * NN_PER_CORE].astype(np.float16)
        )
        in_maps.append({"x": shard, "wqkT": wqkT, "wvT": wvT, "wprojT": wprojT})

    kw = {}
    if _trace:
        import tempfile
        kw = dict(trace=True, tmpdir=tempfile.mkdtemp(prefix="attn2_trace_"))
    res = run_bass_kernel_spmd(nc, in_maps, list(range(N_CORES)), **kw)
    LAST_RESULT["res"] = res
    LAST_RESULT["tmpdir"] = kw.get("tmpdir")
    out = np.empty((n, c, t, vv), dtype=np.float32)
    for core in range(N_CORES):
        out[core * NN_PER_CORE:(core + 1) * NN_PER_CORE] = res.results[core]["y"]
    return out



# revision 7
# speedup vs baseline: 1.2083x; 1.2083x over previous
"""Trainium2 Bass kernel for nn_Attention2 (dense transformer block with
softmax over the heads axis).

Computation per (n, t) batch b (B = n*t = 4096 total, X_b = x[n,:,t,:].T is
[vv=25, c=512]):
    qkv = X_b @ w_qkv.T, split into q,k,v heads [h=8, 25, hd=64]
    s[h,i,j] = (q[h,i,:] . k[h,j,:]) / 8      (scale folded into w_q on host)
    p = softmax over h (axis 0)
    o[h,i,:] = sum_j p[h,i,j] v[h,j,:]  -> [25, 512] -> @ w_proj.T
    out[n,:,t,:] = result.T

Sharding: data-parallel over n, 2 n-values (512 batches) per core, 8 cores.

v2 structure: two-stage software pipeline over groups of TG=16 batches.
Group g's attention core (small packed matmuls + softmax chain) is emitted
interleaved with group g+1's qkT GEMM chunks so the PE never idles long
enough for the HAM clock gate to re-throttle it to 1.2 GHz (the v1 kernel
lost ~40% of its runtime to HAM oscillation, one cold phase per group).
Engine balance: DVE does qkT/po evacuation + reduce, ACT does exp/v2/fin
evacuation, GpSimd does the softmax reciprocal + divide.
"""
import numpy as np
import concourse.bass as bass
import concourse.mybir as mybir
import concourse.tile as tile
from concourse.bass_utils import run_bass_kernel_spmd
from concourse.vector_clock import ScopedClock, VectorClock

F32 = mybir.dt.float32
F16 = mybir.dt.float16

N_CORES = 8
NN_PER_CORE = 2        # n values per core
T = 256
VV = 25
C = 512
H = 8
HD = 64
TG = 16                # t values (batches) per group
NGROUPS = NN_PER_CORE * (T // TG)   # 32 groups per core
NB = TG * VV           # 400 moving columns per group


def _split_drain_and_barrier(self, tick_clock, wait_clock):
    # walrus caps sync-wait commands at 1 for CTRL_NO; split the kernel-tail
    # drain into one drain per pending proc.
    vc = tick_clock.global_clock
    n = len(vc)
    for i in range(n):
        if vc[i] == 0:
            continue
        sub = VectorClock([vc[j] if j == i else 0 for j in range(n)])
        d = self.nc.sync.drain()
        wait_clock.add_sem_waits(d.ins, ScopedClock({None: sub}))
    self.nc.all_engine_barrier()
    assert self.sems is not None
    popped = self.nc._tile_sem_poison_stack.pop()
    assert popped is self._sem_poison
    self.nc.clear_and_free_semaphores(list(self.sems.allocated().values()))
    self.nc.all_engine_barrier()


tile.TileContext._drain_and_barrier = _split_drain_and_barrier


def split_excess_waits(nc, limit=1):
    """walrus codegen allows very few sync-wait commands per instruction
    (1 for matmul/drain/DMA structs).  Move excess waits onto same-engine
    NoOp carriers inserted just before the instruction — same semantics,
    since each engine executes its queue in order."""
    k = 0
    for fn in nc.m.functions:
        for bb in fn.blocks:
            out = []
            for ins in bb.instructions:
                si = ins.sync_info
                waits = list(si.on_wait) if si is not None and si.on_wait else []
                if len(waits) > limit:
                    keep = waits[-limit:]
                    for w in waits[:-limit]:
                        nop = mybir.InstNoOp(
                            name=f"WC-{k}", ins=[], outs=[], engine=ins.engine
                        )
                        k += 1
                        nop.sync_info = mybir.SyncInfo(on_wait=[w], on_update=[])
                        out.append(nop)
                    si.on_wait = keep
                out.append(ins)
            bb.instructions[:] = out
    return k


def build_nc():
    nc = bass.Bass()
    X = nc.declare_dram_parameter("x", [NN_PER_CORE, C, T, VV], F16, isOutput=False)
    WQK = nc.declare_dram_parameter("wqkT", [C, 2 * C], F16, isOutput=False)
    WV = nc.declare_dram_parameter("wvT", [C, C], F16, isOutput=False)
    WP = nc.declare_dram_parameter("wprojT", [C, C], F16, isOutput=False)
    Y = nc.declare_dram_parameter("y", [NN_PER_CORE, C, T, VV], F32, isOutput=True)

    with tile.TileContext(nc) as tc:
        with (
            tc.tile_pool(name="consts", bufs=1) as consts,
            tc.tile_pool(name="perg", bufs=2) as perg,
            tc.tile_pool(name="pers", bufs=2) as pers,
            tc.tile_pool(name="pbig", bufs=4, space="PSUM") as pbig,
            tc.tile_pool(name="psmall", bufs=1, space="PSUM") as psmall,
        ):
            # ---- load the fp16 weights ----
            wqk_r, wv_r, wp_r = [], [], []
            for kc in range(4):
                r0 = consts.tile([128, 2 * C], F16, tag=f"wqkr{kc}")
                nc.sync.dma_start(out=r0, in_=WQK[kc * 128:(kc + 1) * 128, :])
                wqk_r.append(r0)
                r1 = consts.tile([128, C], F16, tag=f"wvr{kc}")
                nc.sync.dma_start(out=r1, in_=WV[kc * 128:(kc + 1) * 128, :])
                wv_r.append(r1)
                r2 = consts.tile([128, C], F16, tag=f"wpr{kc}")
                nc.sync.dma_start(out=r2, in_=WP[kc * 128:(kc + 1) * 128, :])
                wp_r.append(r2)

            # per-group state carried between pipeline stages
            xp_tiles = {}     # g -> [4 chunks]
            qkT_tiles = {}    # g -> [8 chunks of [128, NB] f16]
            oT_tiles = {}     # g -> oT tile

            def emit_x_dma(g):
                nn = g // (T // TG)
                t0 = (g % (T // TG)) * TG
                xp = []
                for kc in range(4):
                    xq = perg.tile([128, TG, 32], F16, tag=f"xp{kc}", bufs=3)
                    nc.sync.dma_start(
                        out=xq[:, :, 0:VV],
                        in_=X[nn, kc * 128:(kc + 1) * 128, t0:t0 + TG, :],
                    )
                    xp.append(xq)
                xp_tiles[g] = xp

            def emit_qkT_chunk(g, m):
                """One output chunk m (128 rows of q^T/k^T) for group g:
                4-matmul kc chain into a pbig bank + one merged DVE cast."""
                xp = xp_tiles[g]
                pq = pbig.tile([128, NB], F32, tag="big")
                for kc in range(4):
                    nc.tensor.matmul(
                        pq[:],
                        wqk_r[kc][:, m * 128:(m + 1) * 128],
                        xp[kc][:, :, 0:VV],
                        start=(kc == 0), stop=(kc == 3),
                    )
                qc = perg.tile([128, NB], F16, tag=f"qkT{m}")
                nc.vector.tensor_copy(qc[:], pq[:])
                qkT_tiles.setdefault(g, [None] * 8)[m] = qc

            def emit_attn_sub(g, sub):
                """Attention core for 4 batches (one sub) of group g."""
                xp = xp_tiles[g]
                qkT = qkT_tiles[g]
                oT = oT_tiles[g]
                bcol0 = sub * 4 * VV

                # v for these 4 batches: [ (b4, j32) , c' ] via x-stationary
                pv = pbig.tile([128, C], F32, tag="big", name="pv")
                for kc in range(4):
                    nc.tensor.matmul(
                        pv[:],
                        xp[kc][:, sub * 4:sub * 4 + 4, :],
                        wv_r[kc][:],
                        start=(kc == 0), stop=(kc == 3),
                    )
                v2 = [pers.tile([64, C], F16, tag=f"v2{q}", name=f"v2{q}") for q in range(2)]
                for q in range(2):
                    nc.scalar.activation(
                        v2[q][:, :], pv[q * 64:(q + 1) * 64, :],
                        mybir.ActivationFunctionType.Copy,
                    )

                # scores s[h, i, j] -> psm[par][ (b4, j32), m, i ]
                psm = [
                    psmall.tile([128, 4, VV], F32, tag=f"psm{par}", name=f"psm{par}")
                    for par in range(2)
                ]
                for h in range(H):
                    m, par, r0 = h // 2, h % 2, (h % 2) * 64
                    for b4 in range(4):
                        bcol = bcol0 + b4 * VV
                        nc.tensor.matmul(
                            psm[par][b4 * 32:b4 * 32 + 25, m, :],
                            qkT[4 + m][r0:r0 + 64, bcol:bcol + VV],
                            qkT[m][r0:r0 + 64, bcol:bcol + VV],
                            start=True, stop=True,
                            tile_position=(r0, b4 * 32),
                        )

                # softmax over heads: e_t[(b4,j32), i, h]
                e_t = perg.tile([128, VV, H], F32, tag="e_t", bufs=3)
                for par in range(2):
                    nc.scalar.activation(
                        e_t[:, :, par::2],
                        psm[par][:].rearrange("p m i -> p i m"),
                        mybir.ActivationFunctionType.Exp,
                    )
                D = perg.tile([128, VV], F32, tag="D", bufs=3)
                nc.vector.reduce_sum(out=D[:], in_=e_t[:], axis=mybir.AxisListType.X)
                rD = perg.tile([128, VV], F32, tag="rD", bufs=3)
                nc.vector.reciprocal(rD[:], D[:])
                p2 = [pers.tile([64, VV, H], F16, tag=f"p2{q}", name=f"p2{q}") for q in range(2)]
                for q in range(2):
                    nc.vector.tensor_mul(
                        p2[q][:],
                        e_t[q * 64:(q + 1) * 64, :, :],
                        rD[q * 64:(q + 1) * 64, :]
                        .unsqueeze(2).broadcast_to([64, VV, H]),
                    )

                # o^T: po[e][c' , m, q*VV + i] for head h=(2m + (c0//64))
                po = [
                    psmall.tile([128, 4, 2 * VV], F32, tag=f"po{e}", name=f"po{e}")
                    for e in range(2)
                ]
                for b4 in range(4):
                    q, e = b4 // 2, b4 % 2
                    for h in range(H):
                        m, c0 = h // 2, (h % 2) * 64
                        nc.tensor.matmul(
                            po[e][c0:c0 + 64, m, q * VV:(q + 1) * VV],
                            v2[q][e * 32:e * 32 + 25, h * HD:(h + 1) * HD],
                            p2[q][e * 32:e * 32 + 25, :, h],
                            start=True, stop=True,
                            tile_position=(e * 32, c0),
                        )
                for e in range(2):
                    dst = oT[:].rearrange(
                        "p m (b i) -> p m b i", i=VV
                    )[:, :, sub * 4 + e:sub * 4 + e + 3:2, :]
                    nc.vector.tensor_copy(
                        dst, po[e][:].rearrange(
                            "p m (b i) -> p m b i", i=VV
                        )
                    )

            def emit_proj(g):
                nn = g // (T // TG)
                t0 = (g % (T // TG)) * TG
                oT = oT_tiles[g]
                for co in range(4):
                    pf = pbig.tile([128, NB], F32, tag="big")
                    for kc in range(4):
                        nc.tensor.matmul(
                            pf[:],
                            wp_r[kc][:, co * 128:(co + 1) * 128],
                            oT[:, kc, :],
                            start=(kc == 0), stop=(kc == 3),
                        )
                    fin = perg.tile([128, NB], F32, tag=f"fin{co}")
                    nc.scalar.activation(
                        fin[:], pf[:], mybir.ActivationFunctionType.Copy,
                    )
                    nc.sync.dma_start(
                        out=Y[nn, co * 128:(co + 1) * 128, t0:t0 + TG, :],
                        in_=fin[:].rearrange("p (t v) -> p t v", t=TG),
                    )
                del xp_tiles[g], qkT_tiles[g], oT_tiles[g]

            # ---- two-stage pipeline: qkT(g) interleaved with attention(g-1)
            emit_x_dma(0)
            for g in range(NGROUPS + 1):
                if g + 1 <= NGROUPS - 1:
                    emit_x_dma(g + 1)
                if g < NGROUPS:
                    oT_tiles[g] = perg.tile(
                        [128, 4, NB], F16, tag="oT", name="oT"
                    )
                for i in range(4):
                    if g < NGROUPS:
                        emit_qkT_chunk(g, 2 * i)
                        emit_qkT_chunk(g, 2 * i + 1)
                    if g >= 1:
                        emit_attn_sub(g - 1, i)
                if g >= 1:
                    emit_proj(g - 1)
    return nc


LAST_RESULT = {}


def kernel(x: np.ndarray, w_qkv: np.ndarray, w_proj: np.ndarray,
           _trace: bool = False) -> np.ndarray:
    n, c, t, vv = x.shape
    assert (n, c, t, vv) == (16, 512, 256, 25)
    scale = np.float32((c // H) ** -0.5)

    wq = w_qkv[:c] * scale
    wk = w_qkv[c:2 * c]
    wv = w_qkv[2 * c:]
    wqkT = np.ascontiguousarray(np.concatenate([wq, wk], axis=0).T.astype(np.float16))
    wvT = np.ascontiguousarray(wv.T.astype(np.float16))
    wprojT = np.ascontiguousarray(w_proj.T.astype(np.float16))

    nc = build_nc()
    split_excess_waits(nc)
    in_maps = []
    for core in range(N_CORES):
        shard = np.ascontiguousarray(
            x[core * NN_PER_CORE:(core + 1) * NN_PER_CORE].astype(np.float16)
        )
        in_maps.append({"x": shard, "wqkT": wqkT, "wvT": wvT, "wprojT": wprojT})

    kw = {}
    if _trace:
        import tempfile
        kw = dict(trace=True, tmpdir=tempfile.mkdtemp(prefix="attn2_trace_"))
    res = run_bass_kernel_spmd(nc, in_maps, list(range(N_CORES)), **kw)
    LAST_RESULT["res"] = res
    LAST_RESULT["tmpdir"] = kw.get("tmpdir")
    out = np.empty((n, c, t, vv), dtype=np.float32)
    for core in range(N_CORES):
        out[core * NN_PER_CORE:(core + 1) * NN_PER_CORE] = res.results[core]["y"]
    return out


# revision 13
# speedup vs baseline: 1.3201x; 1.0925x over previous
"""Trainium2 Bass kernel for nn_Attention2 (dense transformer block with
softmax over the heads axis).

Computation per (n, t) batch b (B = n*t = 4096 total, X_b = x[n,:,t,:].T is
[vv=25, c=512]):
    qkv = X_b @ w_qkv.T, split into q,k,v heads [h=8, 25, hd=64]
    s[h,i,j] = (q[h,i,:] . k[h,j,:]) / 8      (scale folded into w_q on host)
    p = softmax over h (axis 0)
    o[h,i,:] = sum_j p[h,i,j] v[h,j,:]  -> [25, 512] -> @ w_proj.T
    out[n,:,t,:] = result.T

Sharding: data-parallel over n, 2 n-values (512 batches) per core, 8 cores.

v2 structure: two-stage software pipeline over groups of TG=16 batches.
Group g's attention core (small packed matmuls + softmax chain) is emitted
interleaved with group g+1's qkT GEMM chunks so the PE never idles long
enough for the HAM clock gate to re-throttle it to 1.2 GHz (the v1 kernel
lost ~40% of its runtime to HAM oscillation, one cold phase per group).
Engine balance: DVE does qkT/po evacuation + reduce, ACT does exp/v2/fin
evacuation, GpSimd does the softmax reciprocal + divide.
"""
import numpy as np
import concourse.bass as bass
import concourse.mybir as mybir
import concourse.tile as tile
from concourse.bass_utils import run_bass_kernel_spmd
from concourse.vector_clock import ScopedClock, VectorClock

F32 = mybir.dt.float32
F16 = mybir.dt.float16

N_CORES = 8
NN_PER_CORE = 2        # n values per core
T = 256
VV = 25
C = 512
H = 8
HD = 64
TG = 16                # t values (batches) per group
NGROUPS = NN_PER_CORE * (T // TG)   # 32 groups per core
NB = TG * VV           # 400 moving columns per group


def _split_drain_and_barrier(self, tick_clock, wait_clock):
    # walrus caps sync-wait commands at 1 for CTRL_NO; split the kernel-tail
    # drain into one drain per pending proc.
    vc = tick_clock.global_clock
    n = len(vc)
    for i in range(n):
        if vc[i] == 0:
            continue
        sub = VectorClock([vc[j] if j == i else 0 for j in range(n)])
        d = self.nc.sync.drain()
        wait_clock.add_sem_waits(d.ins, ScopedClock({None: sub}))
    self.nc.all_engine_barrier()
    assert self.sems is not None
    popped = self.nc._tile_sem_poison_stack.pop()
    assert popped is self._sem_poison
    self.nc.clear_and_free_semaphores(list(self.sems.allocated().values()))
    self.nc.all_engine_barrier()


tile.TileContext._drain_and_barrier = _split_drain_and_barrier


def split_excess_waits(nc, limit=1):
    """walrus codegen allows very few sync-wait commands per instruction
    (1 for matmul/drain/DMA structs).  Move excess waits onto same-engine
    NoOp carriers inserted just before the instruction — same semantics,
    since each engine executes its queue in order."""
    k = 0
    for fn in nc.m.functions:
        for bb in fn.blocks:
            out = []
            for ins in bb.instructions:
                si = ins.sync_info
                waits = list(si.on_wait) if si is not None and si.on_wait else []
                if len(waits) > limit:
                    keep = waits[-limit:]
                    for w in waits[:-limit]:
                        nop = mybir.InstNoOp(
                            name=f"WC-{k}", ins=[], outs=[], engine=ins.engine
                        )
                        k += 1
                        nop.sync_info = mybir.SyncInfo(on_wait=[w], on_update=[])
                        out.append(nop)
                    si.on_wait = keep
                out.append(ins)
            bb.instructions[:] = out
    return k


def build_nc():
    nc = bass.Bass()
    X = nc.declare_dram_parameter("x", [NN_PER_CORE, C, T, VV], F16, isOutput=False)
    WQK = nc.declare_dram_parameter("wqkT", [C, 2 * C], F16, isOutput=False)
    WV = nc.declare_dram_parameter("wvT", [C, C], F16, isOutput=False)
    WP = nc.declare_dram_parameter("wprojT", [C, C], F16, isOutput=False)
    Y = nc.declare_dram_parameter("y", [NN_PER_CORE, C, T, VV], F32, isOutput=True)

    with tile.TileContext(nc) as tc:
        with (
            tc.tile_pool(name="consts", bufs=1) as consts,
            tc.tile_pool(name="perg", bufs=2) as perg,
            tc.tile_pool(name="pers", bufs=3) as pers,
            tc.tile_pool(name="pbig", bufs=4, space="PSUM") as pbig,
            tc.tile_pool(name="psmall", bufs=1, space="PSUM") as psmall,
        ):
            # ---- load the fp16 weights ----
            wqk_r, wv_r, wp_r = [], [], []
            for kc in range(4):
                r0 = consts.tile([128, 2 * C], F16, tag=f"wqkr{kc}")
                nc.sync.dma_start(out=r0, in_=WQK[kc * 128:(kc + 1) * 128, :])
                wqk_r.append(r0)
                r1 = consts.tile([128, C], F16, tag=f"wvr{kc}")
                nc.sync.dma_start(out=r1, in_=WV[kc * 128:(kc + 1) * 128, :])
                wv_r.append(r1)
                r2 = consts.tile([128, C], F16, tag=f"wpr{kc}")
                nc.sync.dma_start(out=r2, in_=WP[kc * 128:(kc + 1) * 128, :])
                wp_r.append(r2)

            # per-group state carried between pipeline stages
            xp_tiles = {}     # g -> [4 chunks]
            qkT_tiles = {}    # g -> [8 chunks of [128, NB] f16]
            oT_tiles = {}     # g -> oT tile

            def emit_x_dma(g):
                nn = g // (T // TG)
                t0 = (g % (T // TG)) * TG
                xp = []
                for kc in range(4):
                    xq = perg.tile([128, TG, 32], F16, tag=f"xp{kc}", bufs=6)
                    nc.sync.dma_start(
                        out=xq[:, :, 0:VV],
                        in_=X[nn, kc * 128:(kc + 1) * 128, t0:t0 + TG, :],
                    )
                    xp.append(xq)
                xp_tiles[g] = xp

            def emit_qkT_chunk_duo(ga, gb, m):
                """One output chunk m (128 rows of q^T/k^T) for groups ga, gb.
                kc-outer with both groups' matmuls sharing one stationary so
                walrus can amortize the LDWEIGHTS; two pbig banks live."""
                pq = {}
                for g in (ga, gb):
                    pq[g] = pbig.tile([128, NB], F32, tag="big", name=f"pq{g%2}")
                for kc in range(4):
                    for g in (ga, gb):
                        nc.tensor.matmul(
                            pq[g][:],
                            wqk_r[kc][:, m * 128:(m + 1) * 128],
                            xp_tiles[g][kc][:, :, 0:VV],
                            start=(kc == 0), stop=(kc == 3),
                        )
                for g in (ga, gb):
                    qc = perg.tile([128, NB], F16, tag=f"qkT{m}", bufs=4)
                    nc.vector.tensor_copy(qc[:], pq[g][:])
                    qkT_tiles.setdefault(g, [None] * 8)[m] = qc

            attn_state = {}

            def emit_attn_pre(g, sub):
                """v + scores + softmax chain for 4 batches (one sub)."""
                xp = xp_tiles[g]
                qkT = qkT_tiles[g]
                bcol0 = sub * 4 * VV

                # v for these 4 batches: [ (b4, j32) , c' ] via x-stationary
                pv = pbig.tile([128, C], F32, tag="big", name="pv")
                for kc in range(4):
                    nc.tensor.matmul(
                        pv[:],
                        xp[kc][:, sub * 4:sub * 4 + 4, :],
                        wv_r[kc][:],
                        start=(kc == 0), stop=(kc == 3),
                    )
                v2 = [pers.tile([64, C], F16, tag=f"v2{q}", name=f"v2{q}") for q in range(2)]
                for q in range(2):
                    nc.scalar.activation(
                        v2[q][:, :], pv[q * 64:(q + 1) * 64, :],
                        mybir.ActivationFunctionType.Copy,
                    )

                # scores s[h, i, j] -> psm[par][ (b4, j32), m, i ]
                psm = [
                    psmall.tile([128, 4, VV], F32, tag=f"psm{par}", name=f"psm{par}")
                    for par in range(2)
                ]
                for h in range(H):
                    m, par, r0 = h // 2, h % 2, (h % 2) * 64
                    for b4 in range(4):
                        bcol = bcol0 + b4 * VV
                        nc.tensor.matmul(
                            psm[par][b4 * 32:b4 * 32 + 25, m, :],
                            qkT[4 + m][r0:r0 + 64, bcol:bcol + VV],
                            qkT[m][r0:r0 + 64, bcol:bcol + VV],
                            start=True, stop=True,
                            tile_position=(r0, b4 * 32),
                        )

                # softmax over heads: e_t[(b4,j32), i, h]
                e_t = perg.tile([128, VV, H], F32, tag="e_t", bufs=3)
                for par in range(2):
                    nc.scalar.activation(
                        e_t[:, :, par::2],
                        psm[par][:].rearrange("p m i -> p i m"),
                        mybir.ActivationFunctionType.Exp,
                    )
                D = perg.tile([128, VV], F32, tag="D", bufs=3)
                nc.vector.reduce_sum(out=D[:], in_=e_t[:], axis=mybir.AxisListType.X)
                rD = perg.tile([128, VV], F32, tag="rD", bufs=3)
                nc.vector.reciprocal(rD[:], D[:])
                p2 = [pers.tile([64, VV, H], F16, tag=f"p2{q}", name=f"p2{q}") for q in range(2)]
                for q in range(2):
                    nc.vector.tensor_mul(
                        p2[q][:],
                        e_t[q * 64:(q + 1) * 64, :, :],
                        rD[q * 64:(q + 1) * 64, :]
                        .unsqueeze(2).broadcast_to([64, VV, H]),
                    )
                attn_state[(g, sub)] = (v2, p2)

            def emit_attn_po(g, sub):
                """Attention-output matmul wave + oT evacuation (runs one
                sub behind emit_attn_pre so the softmax chain is hidden)."""
                v2, p2 = attn_state.pop((g, sub))
                oT = oT_tiles[g]
                # o^T: po[e][c' , m, q*VV + i] for head h=(2m + (c0//64))
                po = [
                    psmall.tile([128, 4, 2 * VV], F32, tag=f"po{e}", name=f"po{e}")
                    for e in range(2)
                ]
                for b4 in range(4):
                    q, e = b4 // 2, b4 % 2
                    for h in range(H):
                        m, c0 = h // 2, (h % 2) * 64
                        nc.tensor.matmul(
                            po[e][c0:c0 + 64, m, q * VV:(q + 1) * VV],
                            v2[q][e * 32:e * 32 + 25, h * HD:(h + 1) * HD],
                            p2[q][e * 32:e * 32 + 25, :, h],
                            start=True, stop=True,
                            tile_position=(e * 32, c0),
                        )
                for e in range(2):
                    dst = oT[:].rearrange(
                        "p m (b i) -> p m b i", i=VV
                    )[:, :, sub * 4 + e:sub * 4 + e + 3:2, :]
                    nc.vector.tensor_copy(
                        dst, po[e][:].rearrange(
                            "p m (b i) -> p m b i", i=VV
                        )
                    )

            def emit_proj(g):
                nn = g // (T // TG)
                t0 = (g % (T // TG)) * TG
                oT = oT_tiles[g]
                for co in range(4):
                    pf = pbig.tile([128, NB], F32, tag="big")
                    for kc in range(4):
                        nc.tensor.matmul(
                            pf[:],
                            wp_r[kc][:, co * 128:(co + 1) * 128],
                            oT[:, kc, :],
                            start=(kc == 0), stop=(kc == 3),
                        )
                    fin = perg.tile([128, NB], F32, tag=f"fin{co}")
                    nc.scalar.activation(
                        fin[:], pf[:], mybir.ActivationFunctionType.Copy,
                    )
                    nc.sync.dma_start(
                        out=Y[nn, co * 128:(co + 1) * 128, t0:t0 + TG, :],
                        in_=fin[:].rearrange("p (t v) -> p t v", t=TG),
                    )
                del xp_tiles[g], qkT_tiles[g], oT_tiles[g]

            # ---- two-stage pipeline over 2-group blocks: qkT duo for
            # (gg, gg+1) interleaved with attention for (gg-2, gg-1); po
            # waves run one sub behind the softmax chain that feeds them.
            emit_x_dma(0)
            emit_x_dma(1)
            for B in range(NGROUPS // 2 + 1):
                gg = 2 * B
                for g in (gg + 2, gg + 3):
                    if g < NGROUPS:
                        emit_x_dma(g)
                subs = []
                if B >= 1:
                    for g in (gg - 2, gg - 1):
                        oT_tiles[g] = perg.tile(
                            [128, 4, NB], F16, tag="oT", name="oT"
                        )
                        subs += [(g, s) for s in range(4)]
                for i in range(8):
                    if gg < NGROUPS:
                        emit_qkT_chunk_duo(gg, gg + 1, i)
                    if B >= 1:
                        emit_attn_pre(*subs[i])
                        if i >= 1:
                            emit_attn_po(*subs[i - 1])
                        if i == 5:
                            emit_proj(gg - 2)
                if B >= 1:
                    emit_attn_po(*subs[7])
                    emit_proj(gg - 1)
    return nc


LAST_RESULT = {}


def kernel(x: np.ndarray, w_qkv: np.ndarray, w_proj: np.ndarray,
           _trace: bool = False) -> np.ndarray:
    n, c, t, vv = x.shape
    assert (n, c, t, vv) == (16, 512, 256, 25)
    scale = np.float32((c // H) ** -0.5)

    wq = w_qkv[:c] * scale
    wk = w_qkv[c:2 * c]
    wv = w_qkv[2 * c:]
    wqkT = np.ascontiguousarray(np.concatenate([wq, wk], axis=0).T.astype(np.float16))
    wvT = np.ascontiguousarray(wv.T.astype(np.float16))
    wprojT = np.ascontiguousarray(w_proj.T.astype(np.float16))

    nc = build_nc()
    split_excess_waits(nc)
    in_maps = []
    for core in range(N_CORES):
        shard = np.ascontiguousarray(
            x[core * NN_PER_CORE:(core + 1) * NN_PER_CORE].astype(np.float16)
        )
        in_maps.append({"x": shard, "wqkT": wqkT, "wvT": wvT, "wprojT": wprojT})

    kw = {}
    if _trace:
        import tempfile
        kw = dict(trace=True, tmpdir=tempfile.mkdtemp(prefix="attn2_trace_"))
    res = run_bass_kernel_spmd(nc, in_maps, list(range(N_CORES)), **kw)
    LAST_RESULT["res"] = res
    LAST_RESULT["tmpdir"] = kw.get("tmpdir")
    out = np.empty((n, c, t, vv), dtype=np.float32)
    for core in range(N_CORES):
        out[core * NN_PER_CORE:(core + 1) * NN_PER_CORE] = res.results[core]["y"]
    return out


# revision 16
# speedup vs baseline: 1.6070x; 1.2173x over previous
"""Trainium2 Bass kernel for nn_Attention2 (dense transformer block with
softmax over the heads axis).

Computation per (n, t) batch b (B = n*t = 4096 total, X_b = x[n,:,t,:].T is
[vv=25, c=512]):
    qkv = X_b @ w_qkv.T, split into q,k,v heads [h=8, 25, hd=64]
    s[h,i,j] = (q[h,i,:] . k[h,j,:]) / 8      (scale folded into w_q on host)
    p = softmax over h (axis 0)
    o[h,i,:] = sum_j p[h,i,j] v[h,j,:]  -> [25, 512] -> @ w_proj.T
    out[n,:,t,:] = result.T

Sharding: data-parallel over n, 2 n-values (512 batches) per core, 8 cores.

v2 structure: two-stage software pipeline over groups of TG=16 batches.
Group g's attention core (small packed matmuls + softmax chain) is emitted
interleaved with group g+1's qkT GEMM chunks so the PE never idles long
enough for the HAM clock gate to re-throttle it to 1.2 GHz (the v1 kernel
lost ~40% of its runtime to HAM oscillation, one cold phase per group).
Engine balance: DVE does qkT/po evacuation + reduce, ACT does exp/v2/fin
evacuation, GpSimd does the softmax reciprocal + divide.
"""
import numpy as np
import concourse.bass as bass
import concourse.mybir as mybir
import concourse.tile as tile
from concourse.bass_utils import run_bass_kernel_spmd
from concourse.vector_clock import ScopedClock, VectorClock

F32 = mybir.dt.float32
F16 = mybir.dt.float16

N_CORES = 8
NN_PER_CORE = 2        # n values per core
T = 256
VV = 25
C = 512
H = 8
HD = 64
TG = 16                # t values (batches) per group
NGROUPS = NN_PER_CORE * (T // TG)   # 32 groups per core
NB = TG * VV           # 400 moving columns per group


def _split_drain_and_barrier(self, tick_clock, wait_clock):
    # walrus caps sync-wait commands at 1 for CTRL_NO; split the kernel-tail
    # drain into one drain per pending proc.
    vc = tick_clock.global_clock
    n = len(vc)
    for i in range(n):
        if vc[i] == 0:
            continue
        sub = VectorClock([vc[j] if j == i else 0 for j in range(n)])
        d = self.nc.sync.drain()
        wait_clock.add_sem_waits(d.ins, ScopedClock({None: sub}))
    self.nc.all_engine_barrier()
    assert self.sems is not None
    popped = self.nc._tile_sem_poison_stack.pop()
    assert popped is self._sem_poison
    self.nc.clear_and_free_semaphores(list(self.sems.allocated().values()))
    self.nc.all_engine_barrier()


tile.TileContext._drain_and_barrier = _split_drain_and_barrier


def split_excess_waits(nc, limit=1):
    """walrus codegen allows very few sync-wait commands per instruction
    (1 for matmul/drain/DMA structs).  Move excess waits onto same-engine
    NoOp carriers inserted just before the instruction — same semantics,
    since each engine executes its queue in order."""
    k = 0
    for fn in nc.m.functions:
        for bb in fn.blocks:
            out = []
            for ins in bb.instructions:
                si = ins.sync_info
                waits = list(si.on_wait) if si is not None and si.on_wait else []
                if len(waits) > limit:
                    keep = waits[-limit:]
                    for w in waits[:-limit]:
                        nop = mybir.InstNoOp(
                            name=f"WC-{k}", ins=[], outs=[], engine=ins.engine
                        )
                        k += 1
                        nop.sync_info = mybir.SyncInfo(on_wait=[w], on_update=[])
                        out.append(nop)
                    si.on_wait = keep
                out.append(ins)
            bb.instructions[:] = out
    return k


def mark_dup_ldweights(nc):
    """Experimental: when two adjacent matmuls use the identical stationary
    operand (our duo-amortized qkT chunks), mark the second one
    ldweights=True to ask walrus to skip re-emitting its LDWEIGHTS.
    Validated end-to-end by the rel-err check — if the flag means something
    else, the result or the LDWEIGHTS count in the trace will say so."""
    n = 0
    for fn in nc.m.functions:
        for bb in fn.blocks:
            prev = None
            for ins in bb.instructions:
                if isinstance(ins, mybir.InstMatmult):
                    if (
                        prev is not None
                        and str(ins.ins[1]) == str(prev.ins[1])
                        and ins.tile_position == prev.tile_position
                        and not ins.is_transpose
                    ):
                        ins.ldweights = True
                        n += 1
                    prev = ins
    return n


def build_nc():
    nc = bass.Bass()
    X = nc.declare_dram_parameter("x", [NN_PER_CORE, C, T, VV], F16, isOutput=False)
    WQK = nc.declare_dram_parameter("wqkT", [C, 2 * C], F16, isOutput=False)
    WV = nc.declare_dram_parameter("wvT", [C, C], F16, isOutput=False)
    WP = nc.declare_dram_parameter("wprojT", [C, C], F16, isOutput=False)
    Y = nc.declare_dram_parameter("y", [NN_PER_CORE, C, T, VV], F32, isOutput=True)

    with tile.TileContext(nc) as tc:
        with (
            tc.tile_pool(name="consts", bufs=1) as consts,
            tc.tile_pool(name="perg", bufs=2) as perg,
            tc.tile_pool(name="pers", bufs=3) as pers,
            tc.tile_pool(name="pbig", bufs=4, space="PSUM") as pbig,
            tc.tile_pool(name="psmall", bufs=1, space="PSUM") as psmall,
        ):
            # ---- load the fp16 weights ----
            wqk_r, wv_r, wp_r = [], [], []
            for kc in range(4):
                r0 = consts.tile([128, 2 * C], F16, tag=f"wqkr{kc}")
                nc.sync.dma_start(out=r0, in_=WQK[kc * 128:(kc + 1) * 128, :])
                wqk_r.append(r0)
                r1 = consts.tile([128, C], F16, tag=f"wvr{kc}")
                nc.sync.dma_start(out=r1, in_=WV[kc * 128:(kc + 1) * 128, :])
                wv_r.append(r1)
                r2 = consts.tile([128, C], F16, tag=f"wpr{kc}")
                nc.sync.dma_start(out=r2, in_=WP[kc * 128:(kc + 1) * 128, :])
                wp_r.append(r2)

            # per-group state carried between pipeline stages
            xp_tiles = {}     # g -> [4 chunks]
            qkT_tiles = {}    # g -> [8 chunks of [128, NB] f16]
            oT_tiles = {}     # g -> oT tile

            def emit_x_dma(g):
                nn = g // (T // TG)
                t0 = (g % (T // TG)) * TG
                xp = []
                for kc in range(4):
                    xq = perg.tile([128, TG, 32], F16, tag=f"xp{kc}", bufs=6)
                    nc.sync.dma_start(
                        out=xq[:, :, 0:VV],
                        in_=X[nn, kc * 128:(kc + 1) * 128, t0:t0 + TG, :],
                    )
                    xp.append(xq)
                xp_tiles[g] = xp

            def emit_qkT_chunk_duo(ga, gb, m):
                """One output chunk m (128 rows of q^T/k^T) for groups ga, gb.
                kc-outer with both groups' matmuls sharing one stationary so
                walrus can amortize the LDWEIGHTS; two pbig banks live."""
                pq = {}
                for g in (ga, gb):
                    pq[g] = pbig.tile([128, NB], F32, tag="big", name=f"pq{g%2}")
                for kc in range(4):
                    for g in (ga, gb):
                        nc.tensor.matmul(
                            pq[g][:],
                            wqk_r[kc][:, m * 128:(m + 1) * 128],
                            xp_tiles[g][kc][:, :, 0:VV],
                            start=(kc == 0), stop=(kc == 3),
                        )
                for g in (ga, gb):
                    qc = perg.tile([128, NB], F16, tag=f"qkT{m}", bufs=4)
                    nc.vector.tensor_copy(qc[:], pq[g][:])
                    qkT_tiles.setdefault(g, [None] * 8)[m] = qc

            attn_state = {}

            def emit_attn_pre(g, sub):
                """v + scores + softmax chain for 4 batches (one sub)."""
                xp = xp_tiles[g]
                qkT = qkT_tiles[g]
                bcol0 = sub * 4 * VV

                # v for these 4 batches: [ (b4, j32) , c' ] via x-stationary
                pv = pbig.tile([128, C], F32, tag="big", name="pv")
                for kc in range(4):
                    nc.tensor.matmul(
                        pv[:],
                        xp[kc][:, sub * 4:sub * 4 + 4, :],
                        wv_r[kc][:],
                        start=(kc == 0), stop=(kc == 3),
                    )
                v2 = [pers.tile([64, C], F16, tag=f"v2{q}", name=f"v2{q}") for q in range(2)]
                for q in range(2):
                    nc.scalar.activation(
                        v2[q][:, :], pv[q * 64:(q + 1) * 64, :],
                        mybir.ActivationFunctionType.Copy,
                    )

                # scores s[h, i, j] -> psm[par][ (b4, j32), m, i ]
                psm = [
                    psmall.tile([128, 4, VV], F32, tag=f"psm{par}", name=f"psm{par}")
                    for par in range(2)
                ]
                for h in range(H):
                    m, par, r0 = h // 2, h % 2, (h % 2) * 64
                    for b4 in range(4):
                        bcol = bcol0 + b4 * VV
                        nc.tensor.matmul(
                            psm[par][b4 * 32:b4 * 32 + 25, m, :],
                            qkT[4 + m][r0:r0 + 64, bcol:bcol + VV],
                            qkT[m][r0:r0 + 64, bcol:bcol + VV],
                            start=True, stop=True,
                            tile_position=(r0, b4 * 32),
                        )

                # softmax over heads: e_t[(b4,j32), i, h]
                e_t = perg.tile([128, VV, H], F32, tag="e_t", bufs=3)
                for par in range(2):
                    nc.scalar.activation(
                        e_t[:, :, par::2],
                        psm[par][:].rearrange("p m i -> p i m"),
                        mybir.ActivationFunctionType.Exp,
                    )
                D = perg.tile([128, VV], F32, tag="D", bufs=3)
                nc.vector.reduce_sum(out=D[:], in_=e_t[:], axis=mybir.AxisListType.X)
                rD = perg.tile([128, VV], F32, tag="rD", bufs=3)
                nc.vector.reciprocal(rD[:], D[:])
                p2 = [pers.tile([64, VV, H], F16, tag=f"p2{q}", name=f"p2{q}") for q in range(2)]
                for q in range(2):
                    nc.vector.tensor_mul(
                        p2[q][:],
                        e_t[q * 64:(q + 1) * 64, :, :],
                        rD[q * 64:(q + 1) * 64, :]
                        .unsqueeze(2).broadcast_to([64, VV, H]),
                    )
                attn_state[(g, sub)] = (v2, p2)

            def emit_attn_po(g, sub):
                """Attention-output matmul wave + oT evacuation (runs one
                sub behind emit_attn_pre so the softmax chain is hidden)."""
                v2, p2 = attn_state.pop((g, sub))
                oT = oT_tiles[g]
                # o^T: po[e][c' , m, q*VV + i] for head h=(2m + (c0//64))
                po = [
                    psmall.tile([128, 4, 2 * VV], F32, tag=f"po{e}", name=f"po{e}")
                    for e in range(2)
                ]
                for b4 in range(4):
                    q, e = b4 // 2, b4 % 2
                    for h in range(H):
                        m, c0 = h // 2, (h % 2) * 64
                        nc.tensor.matmul(
                            po[e][c0:c0 + 64, m, q * VV:(q + 1) * VV],
                            v2[q][e * 32:e * 32 + 25, h * HD:(h + 1) * HD],
                            p2[q][e * 32:e * 32 + 25, :, h],
                            start=True, stop=True,
                            tile_position=(e * 32, c0),
                        )
                for e in range(2):
                    dst = oT[:].rearrange(
                        "p m (b i) -> p m b i", i=VV
                    )[:, :, sub * 4 + e:sub * 4 + e + 3:2, :]
                    nc.vector.tensor_copy(
                        dst, po[e][:].rearrange(
                            "p m (b i) -> p m b i", i=VV
                        )
                    )

            def emit_proj(g):
                nn = g // (T // TG)
                t0 = (g % (T // TG)) * TG
                oT = oT_tiles[g]
                for co in range(4):
                    pf = pbig.tile([128, NB], F32, tag="big")
                    for kc in range(4):
                        nc.tensor.matmul(
                            pf[:],
                            wp_r[kc][:, co * 128:(co + 1) * 128],
                            oT[:, kc, :],
                            start=(kc == 0), stop=(kc == 3),
                        )
                    fin = perg.tile([128, NB], F32, tag=f"fin{co}")
                    nc.scalar.activation(
                        fin[:], pf[:], mybir.ActivationFunctionType.Copy,
                    )
                    nc.sync.dma_start(
                        out=Y[nn, co * 128:(co + 1) * 128, t0:t0 + TG, :],
                        in_=fin[:].rearrange("p (t v) -> p t v", t=TG),
                    )
                del xp_tiles[g], qkT_tiles[g], oT_tiles[g]

            # ---- two-stage pipeline over 2-group blocks: qkT duo for
            # (gg, gg+1) interleaved with attention for (gg-2, gg-1); po
            # waves run one sub behind the softmax chain that feeds them,
            # and the final po+proj of a block carries into the next block
            # so the PE always has qkT work behind a stalled wave (keeps
            # the HAM clock gate warm across block boundaries).
            emit_x_dma(0)
            emit_x_dma(1)
            pending = None
            for B in range(NGROUPS // 2 + 1):
                gg = 2 * B
                for g in (gg + 2, gg + 3):
                    if g < NGROUPS:
                        emit_x_dma(g)
                subs = []
                if B >= 1:
                    for g in (gg - 2, gg - 1):
                        oT_tiles[g] = perg.tile(
                            [128, 4, NB], F16, tag="oT", name="oT", bufs=3
                        )
                        subs += [(g, s) for s in range(4)]
                for i in range(8):
                    if gg < NGROUPS:
                        emit_qkT_chunk_duo(gg, gg + 1, i)
                    if i == 0 and pending is not None:
                        emit_attn_po(*pending)
                        emit_proj(pending[0])
                        pending = None
                    if B >= 1:
                        emit_attn_pre(*subs[i])
                        if i >= 1:
                            emit_attn_po(*subs[i - 1])
                        if i == 5:
                            emit_proj(gg - 2)
                if B >= 1:
                    pending = subs[7]
            if pending is not None:
                emit_attn_po(*pending)
                emit_proj(pending[0])
    return nc


LAST_RESULT = {}


def kernel(x: np.ndarray, w_qkv: np.ndarray, w_proj: np.ndarray,
           _trace: bool = False) -> np.ndarray:
    n, c, t, vv = x.shape
    assert (n, c, t, vv) == (16, 512, 256, 25)
    scale = np.float32((c // H) ** -0.5)

    wq = w_qkv[:c] * scale
    wk = w_qkv[c:2 * c]
    wv = w_qkv[2 * c:]
    wqkT = np.ascontiguousarray(np.concatenate([wq, wk], axis=0).T.astype(np.float16))
    wvT = np.ascontiguousarray(wv.T.astype(np.float16))
    wprojT = np.ascontiguousarray(w_proj.T.astype(np.float16))

    nc = build_nc()
    import os
    if not os.environ.get("NO_LDW_DEDUP"):
        nmarked = mark_dup_ldweights(nc)
        print(f"mark_dup_ldweights: marked {nmarked} matmuls")
    split_excess_waits(nc)
    in_maps = []
    for core in range(N_CORES):
        shard = np.ascontiguousarray(
            x[core * NN_PER_CORE:(core + 1) * NN_PER_CORE].astype(np.float16)
        )
        in_maps.append({"x": shard, "wqkT": wqkT, "wvT": wvT, "wprojT": wprojT})

    kw = {}
    if _trace:
        import tempfile
        kw = dict(trace=True, tmpdir=tempfile.mkdtemp(prefix="attn2_trace_"))
    res = run_bass_kernel_spmd(nc, in_maps, list(range(N_CORES)), **kw)
    LAST_RESULT["res"] = res
    LAST_RESULT["tmpdir"] = kw.get("tmpdir")
    out = np.empty((n, c, t, vv), dtype=np.float32)
    for core in range(N_CORES):
        out[core * NN_PER_CORE:(core + 1) * NN_PER_CORE] = res.results[core]["y"]
    return out
